# revision 3
# baseline (speedup 1.0000x reference)
"""Block-sparse attention v6: variable group sizes + PE warmup + fine tail.

Device computes block-local num|den (bf16) per
    out_q = (num_q + Vtot - Vb) / (den_q + S - 64);
host applies the corrections and divides.

v6 over v5:
  - group sizes (4, 8x7, 4): a small first group starts the exp pipeline on a
    256-col first slab; a small last group shortens the EV->readout->store
    tail chain.
  - qk slabs aligned to group boundaries, all on the SP queue (ACT-queue DMAs
    delay exp dispatch).
  - PE warmup: a few dependency-free matmuls on a memset tile raise the PE
    p-state before the first real scores arrive.
  - final stores split ((48,60),(60,65)) so only a 640B/partition store waits
    for the last group + den row.
"""

import numpy as np

H, S, D = 16, 4096, 64
HPC = 2
NCORES = 8
NBLK = 64
GROUPS = (4, 8, 8, 8, 8, 8, 8, 8, 4)
SCALE = 0.125
QKSLABS = (4, 16, 16, 12, 16)  # blocks per slab; slab 4 rides the Pool queue
QKPOOL = 4  # index of the slab loaded via gpsimd, covers blocks 36-51
VSLABS = (4, 32, 28)  # v slabs in blocks
OSLABS = ((0, 16), (16, 32), (32, 44), (44, 52), (52, 60), (60, 65))
NWARM = 2

_CACHE = {}
OUT_NAMES = ["out"]


def _build_bass():
    import concourse.bass as bass
    import concourse.bacc as bacc
    import concourse.tile as tile
    from concourse import mybir

    f32 = mybir.dt.float32
    bf16 = mybir.dt.bfloat16
    EXP = mybir.ActivationFunctionType.Exp

    nc = bacc.Bacc(
        "TRN2", target_bir_lowering=False, debug=False, num_devices=NCORES
    )
    qk_d = nc.dram_tensor("qk", [128, 2, S], bf16, kind="ExternalInput")
    v_d = nc.dram_tensor("value", [128, NBLK, D], bf16, kind="ExternalInput")
    o_d = nc.dram_tensor("out", [128, NBLK + 1, D], bf16, kind="ExternalOutput")

    goff = [0]
    for n in GROUPS:
        goff.append(goff[-1] + n)
    assert goff[-1] == NBLK
    NGRP = len(GROUPS)

    with tile.TileContext(nc) as tc:
        with (
            tc.tile_pool(name="consts", bufs=1) as consts,
            tc.tile_pool(name="io", bufs=1) as io,
            tc.tile_pool(name="work", bufs=3) as work,
            tc.tile_pool(name="ps_s", bufs=3, space="PSUM") as ps_s,
            tc.tile_pool(name="ps_n", bufs=4, space="PSUM") as ps_n,
            tc.tile_pool(name="ps_d", bufs=1, space="PSUM") as ps_d,
        ):
            dummy = consts.tile([1, 1], f32, tag="dummy")
            nc.gpsimd.memset(dummy, 0.0)
            nc.scalar.activation(out=dummy, in_=dummy, func=EXP, scale=1.0)
            ones = consts.tile([128, 1], bf16, tag="ones")
            nc.vector.memset(ones, 1.0)

            # PE warmup: raise p-state before real work arrives
            warm = consts.tile([128, 384], bf16, tag="warm")
            nc.vector.memset(warm, 0.0)
            pwarm = ps_n.tile([1, 384], f32, tag="pn", name="pwarm")
            for _ in range(NWARM):
                nc.tensor.matmul(pwarm, ones, warm, start=True, stop=True)

            # block ranges: SP slabs (0-3, 4-19, 20-35, 52-63), Pool slab (36-51)
            qkrange = [(0, 4), (4, 20), (20, 36), (52, 64), (36, 52)]
            qks = [
                io.tile([128, 2, (b - a) * D], bf16, tag=f"qk{s}", name=f"qk{s}")
                for s, (a, b) in enumerate(qkrange)
            ]
            vboff = [0]
            for n in VSLABS:
                vboff.append(vboff[-1] + n)
            assert vboff[-1] == NBLK
            vhs = [
                io.tile([128, n, D], bf16, tag=f"vh{s}", name=f"vh{s}")
                for s, n in enumerate(VSLABS)
            ]
            oh = io.tile([128, NBLK + 1, D], bf16, tag="oh")

            for s in range(4):
                a, b = qkrange[s]
                nc.sync.dma_start(out=qks[s], in_=qk_d[:, :, a * D : b * D])
            # Pool queue: vh0, vh1, the mid qk slab, vh2 (deadline order)
            nc.gpsimd.dma_start(out=vhs[0], in_=v_d[:, vboff[0] : vboff[1], :])
            nc.gpsimd.dma_start(out=vhs[1], in_=v_d[:, vboff[1] : vboff[2], :])
            a, b = qkrange[4]
            nc.gpsimd.dma_start(out=qks[4], in_=qk_d[:, :, a * D : b * D])
            nc.gpsimd.dma_start(out=vhs[2], in_=v_d[:, vboff[2] : vboff[3], :])

            def qkblk(blk):
                for s, (a, b) in enumerate(qkrange):
                    if a <= blk < b:
                        lo = (blk - a) * D
                        return qks[s], slice(lo, lo + D)
                raise AssertionError

            def vblk(b):
                for s in range(len(VSLABS)):
                    if b < vboff[s + 1]:
                        return vhs[s], b - vboff[s]
                raise AssertionError

            den = ps_d.tile([128, NBLK, 1], f32, tag="den")
            sco = {}

            def scores(g):
                n = GROUPS[g]
                pss = ps_s.tile([128, n, D], f32, tag="ps", name=f"ps_{g}")
                sco[g] = pss
                for i in range(n):
                    qk, cols = qkblk(goff[g] + i)
                    for lo, hi in ((0, 64), (64, 128)):
                        nc.tensor.matmul(
                            pss[lo:hi, i, :],
                            qk[lo:hi, 1, cols],
                            qk[lo:hi, 0, cols],
                            start=True,
                            stop=True,
                        )

            nst = [0]

            def store_ready(blocks_done):
                while nst[0] < len(OSLABS):
                    s0, s1 = OSLABS[nst[0]]
                    if min(s1, NBLK) > blocks_done:
                        break
                    eng = nc.scalar if s1 > NBLK else nc.sync
                    eng.dma_start(out=o_d[:, s0:s1, :], in_=oh[:, s0:s1, :])
                    nst[0] += 1

            scores(0)
            scores(1)
            for g in range(NGRP):
                if g + 2 < NGRP:
                    scores(g + 2)
                n = GROUPS[g]
                b0 = goff[g]
                pss = sco.pop(g)
                et = work.tile([128, n, D], bf16, tag="et", name=f"et_{g}")
                nc.scalar.activation(out=et, in_=pss, func=EXP, scale=SCALE)
                num = ps_n.tile([128, n, D], f32, tag="pn", name=f"pn_{g}")
                for i in range(n):
                    b = b0 + i
                    vt, vi = vblk(b)
                    for lo, hi in ((0, 64), (64, 128)):
                        nc.tensor.matmul(
                            num[lo:hi, i, :],
                            et[lo:hi, i, :],
                            vt[lo:hi, vi, :],
                            start=True,
                            stop=True,
                        )
                        nc.tensor.matmul(
                            den[lo:hi, b, :],
                            et[lo:hi, i, :],
                            ones[lo:hi, :],
                            start=True,
                            stop=True,
                        )
                nc.vector.tensor_copy(out=oh[:, b0 : b0 + n, :], in_=num)
                nc.scalar.copy(
                    out=oh[:, NBLK, b0 : b0 + n],
                    in_=den[:, b0 : b0 + n, :].rearrange("p b one -> p (b one)"),
                )
                store_ready(b0 + n)

    nc.compile()
    return nc


def _get_compiled():
    if "nc" not in _CACHE:
        _CACHE["nc"] = _build_bass()
    return _CACHE["nc"]


def make_in_maps(query, key, value):
    import ml_dtypes

    bf16 = ml_dtypes.bfloat16
    q = np.asarray(query).reshape(H, S, D)
    k = np.asarray(key).reshape(H, S, D)
    v = np.asarray(value).reshape(H, S, D)
    in_maps = []
    for c in range(NCORES):
        hs = slice(2 * c, 2 * c + 2)
        qk = np.empty((128, 2, S), dtype=bf16)
        qk[:, 0, :] = q[hs].transpose(0, 2, 1).reshape(128, S).astype(bf16)
        qk[:, 1, :] = k[hs].transpose(0, 2, 1).reshape(128, S).astype(bf16)
        vb = v[hs].reshape(2, NBLK, D, D).transpose(0, 2, 1, 3).reshape(128, NBLK, D)
        in_maps.append(
            {
                "qk": np.ascontiguousarray(qk),
                "value": np.ascontiguousarray(vb.astype(bf16)),
            }
        )
    return in_maps


def run_spmd(in_maps, **kwargs):
    from concourse.bass_utils import run_bass_kernel_spmd

    nc = _get_compiled()
    return run_bass_kernel_spmd(nc, in_maps, core_ids=list(range(NCORES)), **kwargs)


def assemble(res, value):
    """Host correction: out = (num + Vtot - Vb) / (den + S - 64)."""
    v = np.asarray(value, dtype=np.float32).reshape(H, S, D)
    vb = v.reshape(H, NBLK, D, D).sum(axis=2)  # [H, 64, D]
    vtot = vb.sum(axis=1)  # [H, D]
    w = (vtot[:, None, :] - vb).astype(np.float32)  # [H, 64, D]

    out = np.empty((H, S, D), dtype=np.float32)
    for c in range(NCORES):
        o = np.asarray(res.results[c]["out"], dtype=np.float32)  # [128, 65, 64]
        o = o.reshape(2, D, NBLK + 1, D)
        for hh in range(2):
            h = 2 * c + hh
            num = o[hh, :, 0:NBLK, :] + w[h][None, :, :]  # [r, b, D]
            den = o[hh, :, NBLK, :] + np.float32(S - 64)  # [r, b]
            out[h] = (num / den[:, :, None]).transpose(1, 0, 2).reshape(S, D)
    return out.reshape(1, H, S, D)


def kernel(query: np.ndarray, key: np.ndarray, value: np.ndarray) -> np.ndarray:
    return assemble(run_spmd(make_in_maps(query, key, value)), value)
